# revision 35
# baseline (speedup 1.0000x reference)
"""Trainium2 Bass kernel for bidirectional-NNF patch voting (bds_vote + blend).

v2 design (fp8 + TensorEngine accumulate):
  - Both voting passes become per-target layered gather slabs of 128 votes.
    Targets are globally sorted by pass-2 vote count (desc) and dealt
    round-robin to 8 cores; each core owns 36 tiles of 128 targets.
  - Gather tables hold fp8(ref) rows (512B each, pixel-major).  Each core's
    stream is split into 2 halves with separate first-use-compacted tables so
    indices fit int16 (per-half distinct rows ~25k < 32767).
  - A vote slab [128 votes, 512ch] is accumulated into the target tile's PSUM
    bank by a TensorE matmul with a stationary identity (weight 1 for pass-1
    slabs, 2*I for pass-2, matching wr == 2*ws).  PSUM accumulates fp32.
  - Finished tiles are copied PSUM -> SBUF acc (bf16), then blended with
    f_a (bf16) once the response threshold (min/max AllReduce) is ready.
  - Gathers cycle over 4 SWDGE queues to overlap Q7 descriptor generation
    with SDMA drains.
"""
import numpy as np
import os
import sys
import types

sys.path.insert(0, "/opt/trn_rl_repo")

import ml_dtypes

F8 = ml_dtypes.float8_e4m3
BF16 = ml_dtypes.bfloat16

C, H, W = 512, 192, 192
N = H * W
ALPHA = 0.8
TAU = 0.05
PAD = -1
NCORES = 8
TPC = N // NCORES          # targets per core = 4608
NT = TPC // 128            # target tiles per core = 36
HALF_NT = NT // 2          # tiles per half = 18
KA = 9                     # pass-1 layers (3x3 offsets, invalid -> zero row)
OUTCH = 6                  # output dma chunk: tiles per dma

_D = {}  # module cache for the compiled program


def _knob(name, default):
    v = os.environ.get("BNNF_" + name, "")
    return int(v) if v else default


def _build_pass1_planes(nnf_sr):
    ry = nnf_sr[..., 0].astype(np.int64)
    rx = nnf_sr[..., 1].astype(np.int64)
    planes = np.full((9, N), PAD, np.int32)
    k = 0
    for dy in (-1, 0, 1):
        for dx in (-1, 0, 1):
            gy = ry + dy
            gx = rx + dx
            valid = (gy >= 0) & (gy < H) & (gx >= 0) & (gx < W)
            src = np.where(valid, gy * W + gx, PAD).astype(np.int32)
            plane = np.full((H, W), PAD, np.int32)
            ty0, ty1 = max(dy, 0), H + min(dy, 0)
            tx0, tx1 = max(dx, 0), W + min(dx, 0)
            plane[ty0:ty1, tx0:tx1] = src[ty0 - dy:ty1 - dy, tx0 - dx:tx1 - dx]
            planes[k] = plane.ravel()
            k += 1
    return planes


def _build_pass2_planes(nnf_rs):
    sy = nnf_rs[..., 0].astype(np.int64)
    sx = nnf_rs[..., 1].astype(np.int64)
    tgt_all, src_all = [], []
    ryg, rxg = np.meshgrid(np.arange(H), np.arange(W), indexing="ij")
    for dy in (-1, 0, 1):
        for dx in (-1, 0, 1):
            ty = sy + dy
            tx = sx + dx
            gy = ryg + dy
            gx = rxg + dx
            valid = ((ty >= 0) & (ty < H) & (tx >= 0) & (tx < W) &
                     (gy >= 0) & (gy < H) & (gx >= 0) & (gx < W))
            tgt_all.append((ty * W + tx)[valid])
            src_all.append((gy * W + gx)[valid])
    tgt = np.concatenate(tgt_all)
    src = np.concatenate(src_all)
    order = np.argsort(tgt, kind="stable")
    tgt_s, src_s = tgt[order], src[order]
    counts = np.bincount(tgt_s, minlength=N)
    starts = np.concatenate(([0], np.cumsum(counts)[:-1]))
    rank = np.arange(len(tgt_s)) - starts[tgt_s]
    K2 = int(counts.max())
    planes = np.full((K2, N), PAD, np.int32)
    planes[rank, tgt_s] = src_s
    return planes, counts


def _wrap_idx(ix):
    """[n] int -> [128, n//16] int16 (wrapped in 16 partitions, replicated x8)."""
    return np.tile(ix.astype(np.int16).reshape(-1, 16).T, (8, 1))


def _make_plan(KB, maxslab):
    """Shared (all-core) instruction plan.

    Each tile's section is [KA pass-1 slabs, KBp pass-2 slabs] padded to an
    even slab count so DoubleRow matmul pairs never straddle tiles or instrs.
    Returns instrs: list of (half, ns, [(series, tau, start, stop), ...])
    and slab streams per half: slabs[h] = [(tau, series, k), ...].
    """
    assert maxslab % 2 == 0
    KBp = [KB[t] + (KA + KB[t]) % 2 for t in range(NT)]
    slabs = [[], []]
    for h in range(2):
        for tau in range(h * HALF_NT, (h + 1) * HALF_NT):
            for k in range(KA):
                slabs[h].append((tau, 0, k))
            for k in range(KBp[tau]):
                slabs[h].append((tau, 1, k))
    instrs = []
    for h in range(2):
        sl = slabs[h]
        for s0 in range(0, len(sl), maxslab):
            ch = sl[s0:s0 + maxslab]
            mm = []
            for (tau, series, k) in ch:
                start = (series == 0 and k == 0)
                stop = (k == (KBp[tau] - 1)) if series == 1 else (
                    KBp[tau] == 0 and k == KA - 1)
                mm.append((series, tau, start, stop))
            instrs.append((h, len(ch), mm))
    return instrs, slabs


def _prep(ref, nnf_sr, nnf_rs, f_a):
    idxA = _build_pass1_planes(np.asarray(nnf_sr))
    idxB, c2 = _build_pass2_planes(np.asarray(nnf_rs))
    K2 = idxB.shape[0]
    c1 = (idxA != PAD).sum(axis=0)

    refT = np.ascontiguousarray(np.asarray(ref).reshape(C, N).T)   # [N, C]
    ref8 = refT.astype(F8)                                         # [N, C] fp8
    faT = np.asarray(f_a).reshape(C, N).T                          # [N, C]

    gorder = np.argsort(-c2, kind="stable")
    globs = [gorder[c::NCORES] for c in range(NCORES)]

    # shared per-tile pass-2 layer counts (max over cores; counts are desc)
    KB = []
    for tau in range(NT):
        KB.append(int(max(c2[g[tau * 128]] for g in globs)))
    instrs, slabs = _make_plan(KB, _knob("MAXSLAB", 16))

    # per-core, per-half: build idx streams + compacted tables
    in_maps = []
    Vh = [0, 0]
    core_data = []
    for ci in range(NCORES):
        glob = globs[ci]
        halves = []
        for h in range(2):
            rows = np.empty((len(slabs[h]), 128), np.int32)
            for si, (tau, series, k) in enumerate(slabs[h]):
                t = glob[tau * 128:(tau + 1) * 128]
                if series == 0:
                    rows[si] = idxA[k][t]
                else:
                    rows[si] = idxB[k][t] if k < K2 else PAD
            stream = rows.ravel()
            valid = stream != PAD
            vals, firsts = np.unique(stream[valid], return_index=True)
            uniq = vals[np.argsort(firsts)]
            nu = len(uniq)
            assert nu + 1 <= 32767, nu
            lut = np.full(N, PAD, np.int32)
            lut[uniq] = np.arange(nu, dtype=np.int32)
            rem = np.where(valid, lut[np.where(valid, stream, 0)], PAD)
            halves.append((uniq, rem))
            Vh[h] = max(Vh[h], nu + 1)
        core_data.append((glob, halves))

    for ci in range(NCORES):
        glob, halves = core_data[ci]
        im = {}
        blobs = []
        for h in range(2):
            uniq, rem = halves[h]
            # pads -> last (zero) row of the shared-size table
            rem = np.where(rem == PAD, Vh[h] - 1, rem)
            blobs.append(rem)
            tab = np.zeros((Vh[h], C), F8)
            tab[:len(uniq)] = ref8[uniq]
            im["t%d" % h] = tab
        # idx blob in instr order (chunks are consecutive slabs per half)
        parts = []
        off = [0, 0]
        for (h, ns, _) in instrs:
            ix = blobs[h][off[h]:off[h] + ns * 128]
            off[h] += ns * 128
            parts.append(_wrap_idx(ix))
        im["idx"] = np.ascontiguousarray(np.concatenate(parts, axis=1)).astype(np.int16)

        fa_core = faT[glob]                                        # [TPC, C]
        im["fa"] = np.ascontiguousarray(
            fa_core.reshape(NT, 128, C).transpose(1, 0, 2)
        ).reshape(128, NT * C).astype(BF16)

        den = (c1[glob] + 2 * c2[glob]).astype(np.float32)
        winv = np.where(den == 0, np.float32(1.0),
                        1.0 / np.maximum(den, 1)).astype(np.float32)
        im["winv"] = np.ascontiguousarray(winv.reshape(NT, 128).T)  # [128, NT]

        # paired-identity weights for DoubleRow: [I|I], [I|2I], [2I|2I]
        I1 = np.eye(128, dtype=np.float32).astype(F8)
        I2 = (2.0 * np.eye(128, dtype=np.float32)).astype(F8)
        ident = np.zeros((128, 768), F8)
        for si, (wa, wb) in enumerate(((I1, I1), (I1, I2), (I2, I2))):
            ident[:, si * 256:si * 256 + 128] = wa
            ident[:, si * 256 + 128:si * 256 + 256] = wb
        im["ident"] = ident
        in_maps.append(im)

    W_total = sum(ns * 8 for (_, ns, _) in instrs)
    plan = dict(instrs=instrs, V0=Vh[0], V1=Vh[1], W_total=W_total)
    return plan, in_maps, [cd[0] for cd in core_data]


def _build_program(plan):
    from concourse import bacc, bass, mybir, tile

    V0, V1 = plan["V0"], plan["V1"]
    WT = plan["W_total"]
    instrs = plan["instrs"]
    NQ = _knob("QUEUES", 4)
    MAXSLAB = _knob("MAXSLAB", 16)
    STG = _knob("STG", 6)
    NO_DR = _knob("NO_DR", 0)
    OC = _knob("OUT_CH", 2)
    COLL_AT = min(_knob("COLL_AT", 3), max(len(instrs) - 2, 0))
    THRESH_AT = max(COLL_AT + 1, min(_knob("THRESH_AT", 28), len(instrs)))
    SKIP_MM = _knob("SKIP_MM", 0)
    SKIP_COLL = _knob("SKIP_COLL", 0)
    MAX_G = _knob("MAX_G", 10 ** 9)
    nc = bacc.Bacc("TRN2", target_bir_lowering=False, debug=False,
                   num_devices=NCORES, num_swdge_queues=NQ,
                   dynamic_dma_scratch_size=_knob("SCRATCH", 49152))
    dt = mybir.dt
    t0 = nc.dram_tensor("t0", [V0, C], dt.float8e4, kind="ExternalInput").ap()
    t1 = nc.dram_tensor("t1", [V1, C], dt.float8e4, kind="ExternalInput").ap()
    idx = nc.dram_tensor("idx", [128, WT], dt.int16, kind="ExternalInput").ap()
    fa = nc.dram_tensor("fa", [128, NT * C], dt.bfloat16, kind="ExternalInput").ap()
    winv = nc.dram_tensor("winv", [128, NT], dt.float32, kind="ExternalInput").ap()
    ident = nc.dram_tensor("ident", [128, 768], dt.float8e4, kind="ExternalInput").ap()
    out = nc.dram_tensor("out", [128, NT * C], dt.bfloat16, kind="ExternalOutput").ap()
    tabs = [t0, t1]

    with tile.TileContext(nc) as tc:
        with tc.tile_pool(name="sbuf", bufs=1) as pool, \
             tc.tile_pool(name="stg", bufs=STG) as stp, \
             tc.tile_pool(name="bl", bufs=2) as blp, \
             tc.tile_pool(name="och", bufs=2) as ocp, \
             tc.tile_pool(name="dram", bufs=1, space="DRAM") as dpool, \
             tc.tile_pool(name="psum", bufs=8, space="PSUM") as psp:
            idx_sb = pool.tile([128, WT], dt.int16)
            fa_sb = pool.tile([128, NT * C], dt.bfloat16)
            acc = pool.tile([128, NT * C], dt.bfloat16)
            id_sb = pool.tile([128, 768], dt.float8e4)
            winv_sb = pool.tile([128, NT], dt.float32)
            resp = pool.tile([128, NT], dt.float32)
            wt = pool.tile([128, NT], dt.float32)
            sfac = pool.tile([128, NT], dt.float32)
            wt_b = pool.tile([128, NT], dt.bfloat16)
            sfac_b = pool.tile([128, NT], dt.bfloat16)
            red1 = pool.tile([128, 2], dt.float32)
            thrb = pool.tile([128, 2], dt.float32)
            thresh = pool.tile([128, 1], dt.float32)
            sq = pool.tile([128, OUTCH, C], dt.bfloat16)
            flat = pool.tile([1, 256], dt.float32)
            packv = pool.tile([1, 2], dt.float32)

            nc.sync.dma_start(out=idx_sb[:], in_=idx[:])
            nc.sync.dma_start(out=winv_sb[:], in_=winv[:])
            nc.sync.dma_start(out=id_sb[:], in_=ident[:])
            nc.sync.dma_start(out=fa_sb[:], in_=fa[:])
            if SKIP_MM:
                nc.vector.memset(acc[:], 0.0)

            # ---- response = sum_c fa^2 (square + reduce per 6-tile chunk) ----
            for t in range(0, NT, OUTCH):
                nch = min(OUTCH, NT - t)
                src = fa_sb[:, t * C:(t + nch) * C].rearrange(
                    "p (t c) -> p t c", c=C)
                nc.vector.tensor_tensor(sq[:, :nch, :], src, src,
                                        mybir.AluOpType.mult)
                nc.vector.tensor_reduce(resp[:, t:t + nch], sq[:, :nch, :],
                                        mybir.AxisListType.X,
                                        mybir.AluOpType.add)

            # ---- cross-partition min/max -> pack -> (collective later) ----
            nc.vector.tensor_reduce(red1[:, 0:1], resp[:],
                                    mybir.AxisListType.X, mybir.AluOpType.max)
            nc.vector.tensor_reduce(red1[:, 1:2], resp[:],
                                    mybir.AxisListType.X, mybir.AluOpType.min)
            nc.vector.tensor_scalar_mul(red1[:, 1:2], red1[:, 1:2], -1.0)
            nc.sync.dma_start(out=flat[:], in_=red1[:])
            nc.vector.tensor_reduce(
                packv[:], flat[:].rearrange("p (k j) -> p j k", j=2),
                mybir.AxisListType.X, mybir.AluOpType.max)
            cc_in = dpool.tile([1, 2], dt.float32)
            cc_out = dpool.tile([1, 2], dt.float32)
            nc.sync.dma_start(out=cc_in[:], in_=packv[:])

            def emit_collective():
                if SKIP_COLL:
                    nc.sync.dma_start(out=cc_out[:], in_=cc_in[:])
                    return
                nc.gpsimd.collective_compute(
                    "AllReduce", mybir.AluOpType.max,
                    replica_groups=[list(range(NCORES))],
                    ins=[cc_in.opt()], outs=[cc_out.opt()])

            def emit_thresh():
                # broadcast [1,2] -> [128,2] with a 0-stride partition DMA
                nc.sync.dma_start(out=thrb[:],
                                  in_=cc_out[:].partition_broadcast(128))
                tmp1 = blp.tile([128, 1], dt.float32, tag="tmp1")
                nc.vector.tensor_scalar_mul(tmp1[:], thrb[:, 0:1], TAU)
                nc.vector.scalar_tensor_tensor(
                    out=thresh[:], in0=thrb[:, 1:2], scalar=-(1.0 - TAU),
                    in1=tmp1[:],
                    op0=mybir.AluOpType.mult, op1=mybir.AluOpType.add)
                nc.vector.tensor_tensor(wt[:], resp[:],
                                        thresh[:].to_broadcast([128, NT]),
                                        mybir.AluOpType.is_gt)
                nc.vector.tensor_scalar_mul(wt[:], wt[:], ALPHA)
                tmp2 = blp.tile([128, NT], dt.float32, tag="tmp2")
                nc.vector.tensor_scalar(tmp2[:], wt[:], -1.0, 1.0,
                                        mybir.AluOpType.mult, mybir.AluOpType.add)
                nc.vector.tensor_tensor(sfac[:], tmp2[:], winv_sb[:],
                                        mybir.AluOpType.mult)
                nc.vector.tensor_copy(wt_b[:], wt[:])
                nc.vector.tensor_copy(sfac_b[:], sfac[:])

            oc_tiles = {}

            def emit_blend(tau):
                # out_tau = fa_tau * w + acc_tau * sfac
                g = blp.tile([128, C], dt.bfloat16, tag="g")
                t2 = blp.tile([128, C], dt.bfloat16, tag="t2")
                nc.vector.tensor_tensor(
                    g[:], acc[:, tau * C:(tau + 1) * C],
                    sfac_b[:, tau:tau + 1].to_broadcast([128, C]),
                    mybir.AluOpType.mult)
                nc.vector.tensor_tensor(
                    t2[:], fa_sb[:, tau * C:(tau + 1) * C],
                    wt_b[:, tau:tau + 1].to_broadcast([128, C]),
                    mybir.AluOpType.mult)
                ci = tau // OC
                if ci not in oc_tiles:
                    oc_tiles[ci] = ocp.tile([128, OC * C], dt.bfloat16,
                                            tag="oc", name="oc")
                oc = oc_tiles[ci]
                nc.vector.tensor_add(
                    oc[:, (tau % OC) * C:(tau % OC + 1) * C], g[:], t2[:])
                if tau % OC == OC - 1:
                    nc.sync.dma_start(
                        out=out[:, ci * OC * C:(ci + 1) * OC * C],
                        in_=oc[:])
                    del oc_tiles[ci]

            # ---- gather + matmul-accumulate stream ----
            woff = 0
            psum_by_tau = {}
            flushed = []
            blended = set()
            thresh_done = False
            for gi, (h, ns, mm) in enumerate(instrs):
                if gi == COLL_AT:
                    emit_collective()
                if gi == THRESH_AT:
                    emit_thresh()
                    thresh_done = True
                    for tau in flushed:
                        if tau not in blended:
                            emit_blend(tau)
                            blended.add(tau)
                nidx = ns * 128
                wcols = ns * 8
                if gi < MAX_G or not SKIP_MM:
                    stg = stp.tile([128, MAXSLAB, C], dt.float8e4, tag="stage")
                if gi < MAX_G:
                    nc.gpsimd.dma_gather(
                        out_ap=stg[:, :ns, :], in_ap=tabs[h],
                        idxs_ap=idx_sb[:, woff:woff + wcols],
                        num_idxs=nidx, num_idxs_reg=nidx, elem_size=C,
                        single_packet=False, queue_num=gi % NQ)
                woff += wcols
                if SKIP_MM:
                    for j, (series, tau, st, sp) in enumerate(mm):
                        if sp:
                            flushed.append(tau)
                            if thresh_done:
                                emit_blend(tau)
                                blended.add(tau)
                    continue

                def finish(tau):
                    nc.any.tensor_copy(acc[:, tau * C:(tau + 1) * C],
                                       psum_by_tau[tau][:])
                    del psum_by_tau[tau]
                    flushed.append(tau)
                    if thresh_done:
                        emit_blend(tau)
                        blended.add(tau)

                if NO_DR:
                    for j, (series, tau, st, sp) in enumerate(mm):
                        if st:
                            psum_by_tau[tau] = psp.tile(
                                [128, C], dt.float32, tag="acc_ps", name="acc_ps")
                        lt = (id_sb[:, 384:512] if series else id_sb[:, 0:128])
                        nc.tensor.matmul(out=psum_by_tau[tau][:], lhsT=lt,
                                         rhs=stg[:, j:j + 1, :], start=st, stop=sp)
                        if sp:
                            finish(tau)
                else:
                    for j0 in range(0, ns, 2):
                        s0, tau, st, _ = mm[j0]
                        s1, tau1, _, sp = mm[j0 + 1]
                        assert tau == tau1
                        sel = s0 + s1
                        if st:
                            psum_by_tau[tau] = psp.tile(
                                [128, C], dt.float32, tag="acc_ps", name="acc_ps")
                        lt = id_sb[:, sel * 256:(sel + 1) * 256].rearrange(
                            "p (ko m) -> p ko m", m=128)
                        nc.tensor.matmul(
                            out=psum_by_tau[tau][:], lhsT=lt,
                            rhs=stg[:, j0:j0 + 2, :], start=st, stop=sp,
                            perf_mode=mybir.MatmulPerfMode.DoubleRow)
                        if sp:
                            finish(tau)
            if not thresh_done:
                emit_thresh()
                for tau in flushed:
                    if tau not in blended:
                        emit_blend(tau)
                        blended.add(tau)
            assert len(blended) == NT and not psum_by_tau
    nc.compile()
    return nc


def _install_ntff_hook():
    try:
        import antenv
        if "antenv.axon_hooks" not in sys.modules:
            mod = types.ModuleType("antenv.axon_hooks")
            _h = [None]
            mod.set_axon_ntff_profile_hook = lambda h: _h.__setitem__(0, h)
            mod.get_axon_ntff_profile_hook = lambda: _h[0]
            sys.modules["antenv.axon_hooks"] = mod
            antenv.axon_hooks = mod
            from trn_agent_boot.trn_boot import _ntff_profile_via_ctypes
            hook = _ntff_profile_via_ctypes('/opt/axon/libaxon_pjrt.so')
            if hook is not None:
                mod.set_axon_ntff_profile_hook(hook)
    except Exception:
        pass


def kernel(ref, f_a, nnf_sr, nnf_rs, _trace=False):
    from concourse.bass_utils import run_bass_kernel_spmd

    _install_ntff_hook()
    ref = np.asarray(ref)
    f_a = np.asarray(f_a)
    plan, in_maps, globs = _prep(ref, nnf_sr, nnf_rs, f_a)

    key = (plan["V0"], plan["V1"], plan["W_total"],
           tuple((h, ns, tuple(mm)) for (h, ns, mm) in plan["instrs"]),
           tuple(sorted((k, v) for k, v in os.environ.items()
                        if k.startswith("BNNF_"))))
    if _D.get("key") != key:
        _D["nc"] = _build_program(plan)
        _D["key"] = key
    nc = _D["nc"]

    res = run_bass_kernel_spmd(nc, in_maps, list(range(NCORES)), trace=_trace)
    if _trace:
        _D["exec_time_ns"] = res.exec_time_ns

    outT = np.empty((N, C), np.float32)
    for c in range(NCORES):
        blob = res.results[c]["out"].reshape(128, NT, C)
        outT[globs[c]] = blob.transpose(1, 0, 2).reshape(TPC, C).astype(np.float32)
    return np.ascontiguousarray(outT.T).reshape(1, C, H, W).astype(np.float32)


# revision 37
# speedup vs baseline: 1.1340x; 1.1340x over previous
"""Trainium2 Bass kernel for bidirectional-NNF patch voting (bds_vote + blend).

v2 design (fp8 + TensorEngine accumulate):
  - Both voting passes become per-target layered gather slabs of 128 votes.
    Targets are globally sorted by pass-2 vote count (desc) and dealt
    round-robin to 8 cores; each core owns 36 tiles of 128 targets.
  - Gather tables hold fp8(ref) rows (512B each, pixel-major).  Each core's
    stream is split into 2 halves with separate first-use-compacted tables so
    indices fit int16 (per-half distinct rows ~25k < 32767).
  - A vote slab [128 votes, 512ch] is accumulated into the target tile's PSUM
    bank by a TensorE matmul with a stationary identity (weight 1 for pass-1
    slabs, 2*I for pass-2, matching wr == 2*ws).  PSUM accumulates fp32.
  - Finished tiles are copied PSUM -> SBUF acc (bf16), then blended with
    f_a (bf16) once the response threshold (min/max AllReduce) is ready.
  - Gathers cycle over 4 SWDGE queues to overlap Q7 descriptor generation
    with SDMA drains.
"""
import numpy as np
import os
import sys
import types

sys.path.insert(0, "/opt/trn_rl_repo")

import ml_dtypes

F8 = ml_dtypes.float8_e4m3
BF16 = ml_dtypes.bfloat16

C, H, W = 512, 192, 192
N = H * W
ALPHA = 0.8
TAU = 0.05
PAD = -1
NCORES = 8
TPC = N // NCORES          # targets per core = 4608
NT = TPC // 128            # target tiles per core = 36
HALF_NT = NT // 2          # tiles per half = 18
KA = 9                     # pass-1 layers (3x3 offsets, invalid -> zero row)
OUTCH = 6                  # output dma chunk: tiles per dma

_D = {}  # module cache for the compiled program


def _knob(name, default):
    v = os.environ.get("BNNF_" + name, "")
    return int(v) if v else default


def _build_pass1_planes(nnf_sr):
    ry = nnf_sr[..., 0].astype(np.int64)
    rx = nnf_sr[..., 1].astype(np.int64)
    planes = np.full((9, N), PAD, np.int32)
    k = 0
    for dy in (-1, 0, 1):
        for dx in (-1, 0, 1):
            gy = ry + dy
            gx = rx + dx
            valid = (gy >= 0) & (gy < H) & (gx >= 0) & (gx < W)
            src = np.where(valid, gy * W + gx, PAD).astype(np.int32)
            plane = np.full((H, W), PAD, np.int32)
            ty0, ty1 = max(dy, 0), H + min(dy, 0)
            tx0, tx1 = max(dx, 0), W + min(dx, 0)
            plane[ty0:ty1, tx0:tx1] = src[ty0 - dy:ty1 - dy, tx0 - dx:tx1 - dx]
            planes[k] = plane.ravel()
            k += 1
    return planes


def _build_pass2_planes(nnf_rs):
    sy = nnf_rs[..., 0].astype(np.int64)
    sx = nnf_rs[..., 1].astype(np.int64)
    tgt_all, src_all = [], []
    ryg, rxg = np.meshgrid(np.arange(H), np.arange(W), indexing="ij")
    for dy in (-1, 0, 1):
        for dx in (-1, 0, 1):
            ty = sy + dy
            tx = sx + dx
            gy = ryg + dy
            gx = rxg + dx
            valid = ((ty >= 0) & (ty < H) & (tx >= 0) & (tx < W) &
                     (gy >= 0) & (gy < H) & (gx >= 0) & (gx < W))
            tgt_all.append((ty * W + tx)[valid])
            src_all.append((gy * W + gx)[valid])
    tgt = np.concatenate(tgt_all)
    src = np.concatenate(src_all)
    order = np.argsort(tgt, kind="stable")
    tgt_s, src_s = tgt[order], src[order]
    counts = np.bincount(tgt_s, minlength=N)
    starts = np.concatenate(([0], np.cumsum(counts)[:-1]))
    rank = np.arange(len(tgt_s)) - starts[tgt_s]
    K2 = int(counts.max())
    planes = np.full((K2, N), PAD, np.int32)
    planes[rank, tgt_s] = src_s
    return planes, counts


def _wrap_idx(ix):
    """[n] int -> [128, n//16] int16 (wrapped in 16 partitions, replicated x8)."""
    return np.tile(ix.astype(np.int16).reshape(-1, 16).T, (8, 1))


def _make_plan(KB, maxslab):
    """Shared (all-core) instruction plan.

    Each tile's section is [KA pass-1 slabs, KBp pass-2 slabs] padded to an
    even slab count so DoubleRow matmul pairs never straddle tiles or instrs.
    Returns instrs: list of (half, ns, [(series, tau, start, stop), ...])
    and slab streams per half: slabs[h] = [(tau, series, k), ...].
    """
    assert maxslab % 2 == 0
    KBp = [KB[t] + (KA + KB[t]) % 2 for t in range(NT)]
    slabs = [[], []]
    for h in range(2):
        for tau in range(h * HALF_NT, (h + 1) * HALF_NT):
            for k in range(KA):
                slabs[h].append((tau, 0, k))
            for k in range(KBp[tau]):
                slabs[h].append((tau, 1, k))
    instrs = []
    for h in range(2):
        sl = slabs[h]
        for s0 in range(0, len(sl), maxslab):
            ch = sl[s0:s0 + maxslab]
            mm = []
            for (tau, series, k) in ch:
                start = (series == 0 and k == 0)
                stop = (k == (KBp[tau] - 1)) if series == 1 else (
                    KBp[tau] == 0 and k == KA - 1)
                mm.append((series, tau, start, stop))
            instrs.append((h, len(ch), mm))
    return instrs, slabs


def _prep(ref, nnf_sr, nnf_rs, f_a):
    idxA = _build_pass1_planes(np.asarray(nnf_sr))
    idxB, c2 = _build_pass2_planes(np.asarray(nnf_rs))
    K2 = idxB.shape[0]
    c1 = (idxA != PAD).sum(axis=0)

    refT = np.ascontiguousarray(np.asarray(ref).reshape(C, N).T)   # [N, C]
    ref8 = refT.astype(F8)                                         # [N, C] fp8
    faT = np.asarray(f_a).reshape(C, N).T                          # [N, C]

    gorder = np.argsort(-c2, kind="stable")
    globs = [gorder[c::NCORES] for c in range(NCORES)]

    # shared per-tile pass-2 layer counts (max over cores; counts are desc)
    KB = []
    for tau in range(NT):
        KB.append(int(max(c2[g[tau * 128]] for g in globs)))
    instrs, slabs = _make_plan(KB, _knob("MAXSLAB", 16))

    # per-core, per-half: build idx streams + compacted tables
    in_maps = []
    Vh = [0, 0]
    core_data = []
    for ci in range(NCORES):
        glob = globs[ci]
        halves = []
        for h in range(2):
            rows = np.empty((len(slabs[h]), 128), np.int32)
            for si, (tau, series, k) in enumerate(slabs[h]):
                t = glob[tau * 128:(tau + 1) * 128]
                if series == 0:
                    rows[si] = idxA[k][t]
                else:
                    rows[si] = idxB[k][t] if k < K2 else PAD
            stream = rows.ravel()
            valid = stream != PAD
            vals, firsts = np.unique(stream[valid], return_index=True)
            uniq = vals[np.argsort(firsts)]
            nu = len(uniq)
            assert nu + 1 <= 32767, nu
            lut = np.full(N, PAD, np.int32)
            lut[uniq] = np.arange(nu, dtype=np.int32)
            rem = np.where(valid, lut[np.where(valid, stream, 0)], PAD)
            halves.append((uniq, rem))
            Vh[h] = max(Vh[h], nu + 1)
        core_data.append((glob, halves))

    for ci in range(NCORES):
        glob, halves = core_data[ci]
        im = {}
        blobs = []
        for h in range(2):
            uniq, rem = halves[h]
            # pads -> last (zero) row of the shared-size table
            rem = np.where(rem == PAD, Vh[h] - 1, rem)
            blobs.append(rem)
            tab = np.zeros((Vh[h], C), F8)
            tab[:len(uniq)] = ref8[uniq]
            im["t%d" % h] = tab
        # idx blob in instr order (chunks are consecutive slabs per half)
        parts = []
        off = [0, 0]
        for (h, ns, _) in instrs:
            ix = blobs[h][off[h]:off[h] + ns * 128]
            off[h] += ns * 128
            parts.append(_wrap_idx(ix))
        im["idx"] = np.ascontiguousarray(np.concatenate(parts, axis=1)).astype(np.int16)

        fa_core = faT[glob]                                        # [TPC, C]
        im["fa"] = np.ascontiguousarray(
            fa_core.reshape(NT, 128, C).transpose(1, 0, 2)
        ).reshape(128, NT * C).astype(BF16)

        den = (c1[glob] + 2 * c2[glob]).astype(np.float32)
        winv = np.where(den == 0, np.float32(1.0),
                        1.0 / np.maximum(den, 1)).astype(np.float32)
        im["winv"] = np.ascontiguousarray(winv.reshape(NT, 128).T)  # [128, NT]

        # paired-identity weights for DoubleRow: [I|I], [I|2I], [2I|2I]
        I1 = np.eye(128, dtype=np.float32).astype(F8)
        I2 = (2.0 * np.eye(128, dtype=np.float32)).astype(F8)
        ident = np.zeros((128, 768), F8)
        for si, (wa, wb) in enumerate(((I1, I1), (I1, I2), (I2, I2))):
            ident[:, si * 256:si * 256 + 128] = wa
            ident[:, si * 256 + 128:si * 256 + 256] = wb
        im["ident"] = ident
        in_maps.append(im)

    W_total = sum(ns * 8 for (_, ns, _) in instrs)
    plan = dict(instrs=instrs, V0=Vh[0], V1=Vh[1], W_total=W_total)
    return plan, in_maps, [cd[0] for cd in core_data]


def _build_program(plan):
    from concourse import bacc, bass, mybir, tile

    V0, V1 = plan["V0"], plan["V1"]
    WT = plan["W_total"]
    instrs = plan["instrs"]
    NQ = _knob("QUEUES", 4)
    MAXSLAB = _knob("MAXSLAB", 16)
    STG = _knob("STG", 6)
    NO_DR = _knob("NO_DR", 0)
    OC = _knob("OUT_CH", 6)
    SP = bool(_knob("SP", 0))
    COLL_AT = min(_knob("COLL_AT", 5), max(len(instrs) - 2, 0))
    THRESH_AT = max(COLL_AT + 1, min(_knob("THRESH_AT", 22), len(instrs)))
    SKIP_MM = _knob("SKIP_MM", 0)
    SKIP_COLL = _knob("SKIP_COLL", 0)
    MAX_G = _knob("MAX_G", 10 ** 9)
    nc = bacc.Bacc("TRN2", target_bir_lowering=False, debug=False,
                   num_devices=NCORES, num_swdge_queues=NQ,
                   dynamic_dma_scratch_size=_knob("SCRATCH", 49152))
    dt = mybir.dt
    t0 = nc.dram_tensor("t0", [V0, C], dt.float8e4, kind="ExternalInput").ap()
    t1 = nc.dram_tensor("t1", [V1, C], dt.float8e4, kind="ExternalInput").ap()
    idx = nc.dram_tensor("idx", [128, WT], dt.int16, kind="ExternalInput").ap()
    fa = nc.dram_tensor("fa", [128, NT * C], dt.bfloat16, kind="ExternalInput").ap()
    winv = nc.dram_tensor("winv", [128, NT], dt.float32, kind="ExternalInput").ap()
    ident = nc.dram_tensor("ident", [128, 768], dt.float8e4, kind="ExternalInput").ap()
    out = nc.dram_tensor("out", [128, NT * C], dt.bfloat16, kind="ExternalOutput").ap()
    tabs = [t0, t1]

    with tile.TileContext(nc) as tc:
        with tc.tile_pool(name="sbuf", bufs=1) as pool, \
             tc.tile_pool(name="stg", bufs=STG) as stp, \
             tc.tile_pool(name="bl", bufs=2) as blp, \
             tc.tile_pool(name="och", bufs=2) as ocp, \
             tc.tile_pool(name="dram", bufs=1, space="DRAM") as dpool, \
             tc.tile_pool(name="psum", bufs=8, space="PSUM") as psp:
            idx_sb = pool.tile([128, WT], dt.int16)
            fa_sb = pool.tile([128, NT * C], dt.bfloat16)
            acc = pool.tile([128, NT * C], dt.bfloat16)
            id_sb = pool.tile([128, 768], dt.float8e4)
            winv_sb = pool.tile([128, NT], dt.float32)
            resp = pool.tile([128, NT], dt.float32)
            wt = pool.tile([128, NT], dt.float32)
            sfac = pool.tile([128, NT], dt.float32)
            wt_b = pool.tile([128, NT], dt.bfloat16)
            sfac_b = pool.tile([128, NT], dt.bfloat16)
            red1 = pool.tile([128, 2], dt.float32)
            thrb = pool.tile([128, 2], dt.float32)
            thresh = pool.tile([128, 1], dt.float32)
            sq = pool.tile([128, OUTCH, C], dt.bfloat16)
            flat = pool.tile([1, 256], dt.float32)
            packv = pool.tile([1, 2], dt.float32)

            nc.sync.dma_start(out=idx_sb[:], in_=idx[:])
            nc.sync.dma_start(out=winv_sb[:], in_=winv[:])
            nc.sync.dma_start(out=id_sb[:], in_=ident[:])
            nc.sync.dma_start(out=fa_sb[:], in_=fa[:])
            if SKIP_MM:
                nc.vector.memset(acc[:], 0.0)

            # ---- response = sum_c fa^2 (square + reduce per 6-tile chunk) ----
            for t in range(0, NT, OUTCH):
                nch = min(OUTCH, NT - t)
                src = fa_sb[:, t * C:(t + nch) * C].rearrange(
                    "p (t c) -> p t c", c=C)
                nc.vector.tensor_tensor(sq[:, :nch, :], src, src,
                                        mybir.AluOpType.mult)
                nc.vector.tensor_reduce(resp[:, t:t + nch], sq[:, :nch, :],
                                        mybir.AxisListType.X,
                                        mybir.AluOpType.add)

            # ---- cross-partition min/max -> pack -> (collective later) ----
            nc.vector.tensor_reduce(red1[:, 0:1], resp[:],
                                    mybir.AxisListType.X, mybir.AluOpType.max)
            nc.vector.tensor_reduce(red1[:, 1:2], resp[:],
                                    mybir.AxisListType.X, mybir.AluOpType.min)
            nc.vector.tensor_scalar_mul(red1[:, 1:2], red1[:, 1:2], -1.0)
            nc.sync.dma_start(out=flat[:], in_=red1[:])
            nc.vector.tensor_reduce(
                packv[:], flat[:].rearrange("p (k j) -> p j k", j=2),
                mybir.AxisListType.X, mybir.AluOpType.max)
            cc_in = dpool.tile([1, 2], dt.float32)
            cc_out = dpool.tile([1, 2], dt.float32)
            nc.sync.dma_start(out=cc_in[:], in_=packv[:])

            def emit_collective():
                if SKIP_COLL:
                    nc.sync.dma_start(out=cc_out[:], in_=cc_in[:])
                    return
                nc.gpsimd.collective_compute(
                    "AllReduce", mybir.AluOpType.max,
                    replica_groups=[list(range(NCORES))],
                    ins=[cc_in.opt()], outs=[cc_out.opt()])

            def emit_thresh():
                # broadcast [1,2] -> [128,2] with a 0-stride partition DMA
                nc.sync.dma_start(out=thrb[:],
                                  in_=cc_out[:].partition_broadcast(128))
                tmp1 = blp.tile([128, 1], dt.float32, tag="tmp1")
                nc.vector.tensor_scalar_mul(tmp1[:], thrb[:, 0:1], TAU)
                nc.vector.scalar_tensor_tensor(
                    out=thresh[:], in0=thrb[:, 1:2], scalar=-(1.0 - TAU),
                    in1=tmp1[:],
                    op0=mybir.AluOpType.mult, op1=mybir.AluOpType.add)
                nc.vector.tensor_tensor(wt[:], resp[:],
                                        thresh[:].to_broadcast([128, NT]),
                                        mybir.AluOpType.is_gt)
                nc.vector.tensor_scalar_mul(wt[:], wt[:], ALPHA)
                tmp2 = blp.tile([128, NT], dt.float32, tag="tmp2")
                nc.vector.tensor_scalar(tmp2[:], wt[:], -1.0, 1.0,
                                        mybir.AluOpType.mult, mybir.AluOpType.add)
                nc.vector.tensor_tensor(sfac[:], tmp2[:], winv_sb[:],
                                        mybir.AluOpType.mult)
                nc.vector.tensor_copy(wt_b[:], wt[:])
                nc.vector.tensor_copy(sfac_b[:], sfac[:])

            oc_tiles = {}

            def emit_blend(tau):
                # out_tau = fa_tau * w + acc_tau * sfac
                g = blp.tile([128, C], dt.bfloat16, tag="g")
                t2 = blp.tile([128, C], dt.bfloat16, tag="t2")
                nc.vector.tensor_tensor(
                    g[:], acc[:, tau * C:(tau + 1) * C],
                    sfac_b[:, tau:tau + 1].to_broadcast([128, C]),
                    mybir.AluOpType.mult)
                nc.vector.tensor_tensor(
                    t2[:], fa_sb[:, tau * C:(tau + 1) * C],
                    wt_b[:, tau:tau + 1].to_broadcast([128, C]),
                    mybir.AluOpType.mult)
                ci = tau // OC
                if ci not in oc_tiles:
                    oc_tiles[ci] = ocp.tile([128, OC * C], dt.bfloat16,
                                            tag="oc", name="oc")
                oc = oc_tiles[ci]
                nc.vector.tensor_add(
                    oc[:, (tau % OC) * C:(tau % OC + 1) * C], g[:], t2[:])
                if tau % OC == OC - 1:
                    nc.sync.dma_start(
                        out=out[:, ci * OC * C:(ci + 1) * OC * C],
                        in_=oc[:])
                    del oc_tiles[ci]

            # ---- gather + matmul-accumulate stream ----
            woff = 0
            psum_by_tau = {}
            flushed = []
            blended = set()
            thresh_done = False
            for gi, (h, ns, mm) in enumerate(instrs):
                if gi == COLL_AT:
                    emit_collective()
                if gi == THRESH_AT:
                    emit_thresh()
                    thresh_done = True
                    for tau in flushed:
                        if tau not in blended:
                            emit_blend(tau)
                            blended.add(tau)
                nidx = ns * 128
                wcols = ns * 8
                if gi < MAX_G or not SKIP_MM:
                    stg = stp.tile([128, MAXSLAB, C], dt.float8e4, tag="stage")
                if gi < MAX_G:
                    nc.gpsimd.dma_gather(
                        out_ap=stg[:, :ns, :], in_ap=tabs[h],
                        idxs_ap=idx_sb[:, woff:woff + wcols],
                        num_idxs=nidx, num_idxs_reg=nidx, elem_size=C,
                        single_packet=SP, queue_num=gi % NQ)
                woff += wcols
                if SKIP_MM:
                    for j, (series, tau, st, sp) in enumerate(mm):
                        if sp:
                            flushed.append(tau)
                            if thresh_done:
                                emit_blend(tau)
                                blended.add(tau)
                    continue

                def finish(tau):
                    nc.any.tensor_copy(acc[:, tau * C:(tau + 1) * C],
                                       psum_by_tau[tau][:])
                    del psum_by_tau[tau]
                    flushed.append(tau)
                    if thresh_done:
                        emit_blend(tau)
                        blended.add(tau)

                if NO_DR:
                    for j, (series, tau, st, sp) in enumerate(mm):
                        if st:
                            psum_by_tau[tau] = psp.tile(
                                [128, C], dt.float32, tag="acc_ps", name="acc_ps")
                        lt = (id_sb[:, 384:512] if series else id_sb[:, 0:128])
                        nc.tensor.matmul(out=psum_by_tau[tau][:], lhsT=lt,
                                         rhs=stg[:, j:j + 1, :], start=st, stop=sp)
                        if sp:
                            finish(tau)
                else:
                    for j0 in range(0, ns, 2):
                        s0, tau, st, _ = mm[j0]
                        s1, tau1, _, sp = mm[j0 + 1]
                        assert tau == tau1
                        sel = s0 + s1
                        if st:
                            psum_by_tau[tau] = psp.tile(
                                [128, C], dt.float32, tag="acc_ps", name="acc_ps")
                        lt = id_sb[:, sel * 256:(sel + 1) * 256].rearrange(
                            "p (ko m) -> p ko m", m=128)
                        nc.tensor.matmul(
                            out=psum_by_tau[tau][:], lhsT=lt,
                            rhs=stg[:, j0:j0 + 2, :], start=st, stop=sp,
                            perf_mode=mybir.MatmulPerfMode.DoubleRow)
                        if sp:
                            finish(tau)
            if not thresh_done:
                emit_thresh()
                for tau in flushed:
                    if tau not in blended:
                        emit_blend(tau)
                        blended.add(tau)
            assert len(blended) == NT and not psum_by_tau
    nc.compile()
    return nc


def _install_ntff_hook():
    try:
        import antenv
        if "antenv.axon_hooks" not in sys.modules:
            mod = types.ModuleType("antenv.axon_hooks")
            _h = [None]
            mod.set_axon_ntff_profile_hook = lambda h: _h.__setitem__(0, h)
            mod.get_axon_ntff_profile_hook = lambda: _h[0]
            sys.modules["antenv.axon_hooks"] = mod
            antenv.axon_hooks = mod
            from trn_agent_boot.trn_boot import _ntff_profile_via_ctypes
            hook = _ntff_profile_via_ctypes('/opt/axon/libaxon_pjrt.so')
            if hook is not None:
                mod.set_axon_ntff_profile_hook(hook)
    except Exception:
        pass


def kernel(ref, f_a, nnf_sr, nnf_rs, _trace=False):
    from concourse.bass_utils import run_bass_kernel_spmd

    _install_ntff_hook()
    ref = np.asarray(ref)
    f_a = np.asarray(f_a)
    plan, in_maps, globs = _prep(ref, nnf_sr, nnf_rs, f_a)

    key = (plan["V0"], plan["V1"], plan["W_total"],
           tuple((h, ns, tuple(mm)) for (h, ns, mm) in plan["instrs"]),
           tuple(sorted((k, v) for k, v in os.environ.items()
                        if k.startswith("BNNF_"))))
    if _D.get("key") != key:
        _D["nc"] = _build_program(plan)
        _D["key"] = key
    nc = _D["nc"]

    res = run_bass_kernel_spmd(nc, in_maps, list(range(NCORES)), trace=_trace)
    if _trace:
        _D["exec_time_ns"] = res.exec_time_ns

    outT = np.empty((N, C), np.float32)
    for c in range(NCORES):
        blob = res.results[c]["out"].reshape(128, NT, C)
        outT[globs[c]] = blob.transpose(1, 0, 2).reshape(TPC, C).astype(np.float32)
    return np.ascontiguousarray(outT.T).reshape(1, C, H, W).astype(np.float32)
